# revision 15
# baseline (speedup 1.0000x reference)
"""MoE kernel v3: pair-wise F-split (2 cores per expert pair, F/2 each).

Experts are paired largest-with-smallest; the pair's two cores each hold
the F-half of BOTH experts (128 KB/partition, same as v1) and process all
tokens of both experts on their half. Slot capacities are global
(CA = largest expert count, CB = largest count among the 4 "small" slot
experts), so the program is SPMD; per-core data decides which experts a
core serves. Partial outputs (bf16) from the two cores of a pair are
summed on host, then combined/scattered as in v1.

Per-core PE work: (CA + CB) columns x 256 cycles — ~4% less than v1's
2*max_count x 256, with essentially v1's DMA volume.

DRAM layouts per core (FL = F/2 = 2048, FLO = FL/128 = 16):
  x   [n_tiles, 128, KO, CT] bf16  slot-A tiles then slot-B tiles
  w1  [2, 4, 128, KO, 512]   bf16  w1[s,q,p,ko,ff] = w1_{e_s}[ko*128+p, h*FL+q*512+ff]
  w2  [2, 2, 128, 8, D]      bf16  w2[s,b,p,fi,d]  = w2_{e_s}[h*FL+(b*8+fi)*128+p, d]
  b1  [128, 2*FLO]           f32   b1[p, s*FLO+fq] = b1_{e_s}[h*FL+fq*128+p]
  y   [n_tiles, 128, KO, CT] bf16  partial (gelu(x@w1l+b1l) @ w2l)^T
(h = the core's half index within its pair.)
"""

import numpy as np
import ml_dtypes

N_CORES = 8
D = 1024
F = 4096
E = 8
KO = D // 128
FL = F // 2          # 2048 local F columns per core
FLO = FL // 128      # 16 local f-chunks
CT = 512

BF16 = ml_dtypes.bfloat16

_NC_CACHE: dict[tuple, object] = {}
LAST_RESULTS = None


def _cap_tiles(C):
    tiles = []
    off = 0
    while C - off >= CT:
        tiles.append((off, CT))
        off += CT
    if off < C:
        tiles.append((off, C - off))
    return tiles


def _build(CA, CB):
    import concourse.mybir as mybir
    from concourse import bacc
    from concourse.tile import TileContext

    fp32 = mybir.dt.float32
    bf16 = mybir.dt.bfloat16

    spec = [(0, off, tw) for off, tw in _cap_tiles(CA)] + [
        (1, off, tw) for off, tw in _cap_tiles(CB)
    ]
    n_tiles = len(spec)

    nc = bacc.Bacc(
        "TRN2", target_bir_lowering=False, debug=False, num_devices=N_CORES
    )
    x = nc.dram_tensor("x", [n_tiles, 128, KO, CT], bf16, kind="ExternalInput")
    w1 = nc.dram_tensor("w1", [2, 4, 128, KO, 512], bf16, kind="ExternalInput")
    w2 = nc.dram_tensor("w2", [2, 2, 128, 8, D], bf16, kind="ExternalInput")
    b1 = nc.dram_tensor("b1", [128, 2 * FLO], fp32, kind="ExternalInput")
    y = nc.dram_tensor("y", [n_tiles, 128, KO, CT], bf16, kind="ExternalOutput")

    with TileContext(nc) as tc:
        with (
            tc.tile_pool(name="wpool", bufs=1) as wpool,
            tc.tile_pool(name="xpool", bufs=3) as xpool,
            tc.tile_pool(name="hpool", bufs=1) as hpool,
            tc.tile_pool(name="ypool", bufs=4) as ypool,
            tc.tile_pool(name="ph", bufs=3, space="PSUM") as phpool,
            tc.tile_pool(name="py", bufs=3, space="PSUM") as pypool,
        ):
            w1_sb = wpool.tile([128, 2, 4, KO, 512], bf16)
            w2_sb = wpool.tile([128, 2, FLO, D], bf16)
            b1_sb = wpool.tile([128, 2 * FLO], fp32)
            nc.sync.dma_start(b1_sb[:], b1[:])

            x_first = xpool.tile([128, KO, CT], bf16, tag="x_sb")
            nc.sync.dma_start(x_first[:], x[0])
            # Slot A's w1 quarters first (PE starts after 1 MB), then its
            # w2 (mm2 needs it ~30us in), then slot B's weights.
            for q in range(4):
                nc.sync.dma_start(w1_sb[:, 0, q], w1[0, q])
            for b in range(2):
                nc.sync.dma_start(w2_sb[:, 0, b * 8 : (b + 1) * 8, :], w2[0, b])
            for q in range(4):
                nc.sync.dma_start(w1_sb[:, 1, q], w1[1, q])
            for b in range(2):
                nc.sync.dma_start(w2_sb[:, 1, b * 8 : (b + 1) * 8, :], w2[1, b])

            for ti, (s, off, tw) in enumerate(spec):
                if ti == 0:
                    x_sb = x_first
                else:
                    x_sb = xpool.tile([128, KO, CT], bf16, tag="x_sb")
                    nc.sync.dma_start(x_sb[:], x[ti])
                h_sb = hpool.tile([128, FLO, CT], bf16)
                for fo in range(FLO):
                    q, fq = divmod(fo, 4)
                    ph = phpool.tile([128, CT], fp32)
                    for ko in range(KO):
                        nc.tensor.matmul(
                            ph[:, :tw],
                            lhsT=w1_sb[:, s, q, ko, fq * 128 : (fq + 1) * 128],
                            rhs=x_sb[:, ko, :tw],
                            start=(ko == 0),
                            stop=(ko == KO - 1),
                        )
                    nc.scalar.activation(
                        h_sb[:, fo, :tw],
                        ph[:, :tw],
                        mybir.ActivationFunctionType.Gelu,
                        bias=b1_sb[:, s * FLO + fo : s * FLO + fo + 1],
                    )
                for do in range(KO):
                    py = pypool.tile([128, CT], fp32)
                    for fo in range(FLO):
                        nc.tensor.matmul(
                            py[:, :tw],
                            lhsT=w2_sb[:, s, fo, do * 128 : (do + 1) * 128],
                            rhs=h_sb[:, fo, :tw],
                            start=(fo == 0),
                            stop=(fo == FLO - 1),
                        )
                    y_do = ypool.tile([128, CT], bf16, tag="y_do")
                    nc.vector.tensor_copy(y_do[:, :tw], py[:, :tw])
                    # Full-width DMA: contiguous rows (128 descriptors, no
                    # strided slow path); pad columns carry ignored stale
                    # data. Per-do DMAs pipeline under the remaining mm2s,
                    # so the kernel tail only waits on one 128 KB transfer.
                    nc.sync.dma_start(y[ti][:, do, :], y_do[:])

    nc.compile()
    return nc, spec


def kernel(x, gate_w, w1, b1, w2, b2):
    from concourse.bass_utils import run_bass_kernel_spmd

    global LAST_RESULTS

    x = np.asarray(x, dtype=np.float32)
    gate_w = np.asarray(gate_w, dtype=np.float32)
    w1 = np.asarray(w1, dtype=np.float32)
    b1 = np.asarray(b1, dtype=np.float32)
    w2 = np.asarray(w2, dtype=np.float32)
    b2 = np.asarray(b2, dtype=np.float32)

    B, S, Din = x.shape
    assert Din == D and gate_w.shape == (D, E)
    T = B * S
    xf = x.reshape(T, D)

    # ---- Host router + dispatch (as v1) ----
    logits = xf.astype(np.float64) @ gate_w.astype(np.float64)
    idx0 = np.argmax(logits, axis=1)
    rows = np.arange(T)
    v0 = logits[rows, idx0]
    l2 = logits.copy()
    l2[rows, idx0] = -np.inf
    idx1 = np.argmax(l2, axis=1)
    v1_ = l2[rows, idx1]
    e1 = np.exp(v1_ - v0)
    cw0 = 1.0 / (1.0 + e1)
    cw1 = e1 / (1.0 + e1)

    token_ids = []
    combine_w = []
    for e in range(E):
        sel0 = idx0 == e
        sel1 = idx1 == e
        ids = np.nonzero(sel0 | sel1)[0]
        w = np.where(sel0[ids], cw0[ids], cw1[ids])
        token_ids.append(ids)
        combine_w.append(w)

    counts = np.array([len(ids) for ids in token_ids])
    # Pair i-th largest with i-th smallest; slot A = the large expert.
    order = np.argsort(-counts)
    pairs = [(int(order[i]), int(order[E - 1 - i])) for i in range(E // 2)]
    CA = int(max(counts[eA] for eA, _ in pairs))
    CB = int(max(counts[eB] for _, eB in pairs))
    CA += CA & 1
    CB += CB & 1

    if (CA, CB) not in _NC_CACHE:
        _NC_CACHE[(CA, CB)] = _build(CA, CB)
    nc, spec = _NC_CACHE[(CA, CB)]
    n_tiles = len(spec)

    # ---- Per-pair token tiles; per-core weight halves ----
    in_maps = [None] * N_CORES
    pair_x = []
    for pi, (eA, eB) in enumerate(pairs):
        xtiles = np.zeros((n_tiles, 128, KO, CT), dtype=BF16)
        for ti, (s, off, tw) in enumerate(spec):
            e = (eA, eB)[s]
            ids_seg = token_ids[e][off : off + tw]
            w_val = len(ids_seg)
            if w_val == 0:
                continue
            blk = (
                xf[ids_seg].astype(BF16).reshape(w_val, KO, 128).transpose(2, 1, 0)
            )
            xtiles[ti, :, :, :w_val] = blk
        xtiles = np.ascontiguousarray(xtiles)
        pair_x.append(xtiles)
        for h in range(2):
            sl = slice(h * FL, (h + 1) * FL)
            w1c = np.stack(
                [
                    w1[e][:, sl]
                    .reshape(KO, 128, 4, 512)
                    .transpose(2, 1, 0, 3)
                    for e in (eA, eB)
                ]
            ).astype(BF16)  # [2, 4, 128, KO, 512]
            w2c = np.stack(
                [
                    w2[e][sl, :]
                    .reshape(2, 8, 128, D)
                    .transpose(0, 2, 1, 3)
                    for e in (eA, eB)
                ]
            ).astype(BF16)  # [2, 2, 128, 8, D]
            b1c = np.concatenate(
                [b1[e][sl].reshape(FLO, 128).T for e in (eA, eB)], axis=1
            )  # [128, 2*FLO]
            in_maps[2 * pi + h] = {
                "x": xtiles,
                "w1": np.ascontiguousarray(w1c),
                "w2": np.ascontiguousarray(w2c),
                "b1": np.ascontiguousarray(b1c),
            }

    res = run_bass_kernel_spmd(nc, in_maps, core_ids=list(range(N_CORES)))
    LAST_RESULTS = res

    # ---- Host: sum the pair halves, combine, scatter ----
    out = np.zeros((T, D), dtype=np.float32)
    for pi, (eA, eB) in enumerate(pairs):
        ysum = res.results[2 * pi]["y"].astype(np.float32) + res.results[
            2 * pi + 1
        ]["y"].astype(np.float32)
        for ti, (s, off, tw) in enumerate(spec):
            e = (eA, eB)[s]
            ids_seg = token_ids[e][off : off + tw]
            w_val = len(ids_seg)
            if w_val == 0:
                continue
            cw_seg = combine_w[e][off : off + w_val].astype(np.float32)
            yt = ysum[ti, :, :, :w_val].transpose(2, 1, 0).reshape(w_val, D)
            out[ids_seg] += cw_seg[:, None] * (yt + b2[e])

    return out.reshape(B, S, D)


# revision 16
# speedup vs baseline: 1.0097x; 1.0097x over previous
"""MoE kernel v3: pair-wise F-split (2 cores per expert pair, F/2 each).

Experts are paired largest-with-smallest; the pair's two cores each hold
the F-half of BOTH experts (128 KB/partition, same as v1) and process all
tokens of both experts on their half. Slot capacities are global
(CA = largest expert count, CB = largest count among the 4 "small" slot
experts), so the program is SPMD; per-core data decides which experts a
core serves. Partial outputs (bf16) from the two cores of a pair are
summed on host, then combined/scattered as in v1.

Per-core PE work: (CA + CB) columns x 256 cycles — ~4% less than v1's
2*max_count x 256, with essentially v1's DMA volume.

DRAM layouts per core (FL = F/2 = 2048, FLO = FL/128 = 16):
  x   [n_tiles, 128, KO, CT] bf16  slot-A tiles then slot-B tiles
  w1  [2, 4, 128, KO, 512]   bf16  w1[s,q,p,ko,ff] = w1_{e_s}[ko*128+p, h*FL+q*512+ff]
  w2  [2, 2, 128, 8, D]      bf16  w2[s,b,p,fi,d]  = w2_{e_s}[h*FL+(b*8+fi)*128+p, d]
  b1  [128, 2*FLO]           f32   b1[p, s*FLO+fq] = b1_{e_s}[h*FL+fq*128+p]
  y   [n_tiles, 128, KO, CT] bf16  partial (gelu(x@w1l+b1l) @ w2l)^T
(h = the core's half index within its pair.)
"""

import numpy as np
import ml_dtypes

N_CORES = 8
D = 1024
F = 4096
E = 8
KO = D // 128
FL = F // 2          # 2048 local F columns per core
FLO = FL // 128      # 16 local f-chunks
CT = 512

BF16 = ml_dtypes.bfloat16

_NC_CACHE: dict[tuple, object] = {}
LAST_RESULTS = None


def _cap_tiles(C):
    tiles = []
    off = 0
    while C - off >= CT:
        tiles.append((off, CT))
        off += CT
    if off < C:
        tiles.append((off, C - off))
    return tiles


def _build(CA, CB):
    import concourse.mybir as mybir
    from concourse import bacc
    from concourse.tile import TileContext

    fp32 = mybir.dt.float32
    bf16 = mybir.dt.bfloat16

    spec = [(0, off, tw) for off, tw in _cap_tiles(CA)] + [
        (1, off, tw) for off, tw in _cap_tiles(CB)
    ]
    n_tiles = len(spec)

    nc = bacc.Bacc(
        "TRN2", target_bir_lowering=False, debug=False, num_devices=N_CORES
    )
    x = nc.dram_tensor("x", [n_tiles, 128, KO, CT], bf16, kind="ExternalInput")
    w1 = nc.dram_tensor("w1", [2, 4, 128, KO, 512], bf16, kind="ExternalInput")
    w2 = nc.dram_tensor("w2", [2, 2, 128, 8, D], bf16, kind="ExternalInput")
    b1 = nc.dram_tensor("b1", [128, 2 * FLO], fp32, kind="ExternalInput")
    y = nc.dram_tensor("y", [n_tiles, 128, KO, CT], bf16, kind="ExternalOutput")

    with TileContext(nc) as tc:
        with (
            tc.tile_pool(name="wpool", bufs=1) as wpool,
            tc.tile_pool(name="xpool", bufs=3) as xpool,
            tc.tile_pool(name="hpool", bufs=2) as hpool,
            tc.tile_pool(name="ypool", bufs=4) as ypool,
            tc.tile_pool(name="ph", bufs=4, space="PSUM") as phpool,
            tc.tile_pool(name="py", bufs=3, space="PSUM") as pypool,
        ):
            w1_sb = wpool.tile([128, 2, 4, KO, 512], bf16)
            w2_sb = wpool.tile([128, 2, FLO, D], bf16)
            b1_sb = wpool.tile([128, 2 * FLO], fp32)
            nc.sync.dma_start(b1_sb[:], b1[:])

            x_first = xpool.tile([128, KO, CT], bf16, tag="x_sb")
            nc.sync.dma_start(x_first[:], x[0])
            # Slot A's w1 quarters first (PE starts after 1 MB), then its
            # w2 (mm2 needs it ~30us in), then slot B's weights.
            for q in range(4):
                nc.sync.dma_start(w1_sb[:, 0, q], w1[0, q])
            for b in range(2):
                nc.sync.dma_start(w2_sb[:, 0, b * 8 : (b + 1) * 8, :], w2[0, b])
            for q in range(4):
                nc.sync.dma_start(w1_sb[:, 1, q], w1[1, q])
            for b in range(2):
                nc.sync.dma_start(w2_sb[:, 1, b * 8 : (b + 1) * 8, :], w2[1, b])

            for ti, (s, off, tw) in enumerate(spec):
                if ti == 0:
                    x_sb = x_first
                else:
                    x_sb = xpool.tile([128, KO, CT], bf16, tag="x_sb")
                    nc.sync.dma_start(x_sb[:], x[ti])
                h_sb = hpool.tile([128, FLO, CT], bf16)
                for fo in range(FLO):
                    q, fq = divmod(fo, 4)
                    ph = phpool.tile([128, CT], fp32)
                    for ko in range(KO):
                        nc.tensor.matmul(
                            ph[:, :tw],
                            lhsT=w1_sb[:, s, q, ko, fq * 128 : (fq + 1) * 128],
                            rhs=x_sb[:, ko, :tw],
                            start=(ko == 0),
                            stop=(ko == KO - 1),
                        )
                    nc.scalar.activation(
                        h_sb[:, fo, :tw],
                        ph[:, :tw],
                        mybir.ActivationFunctionType.Gelu,
                        bias=b1_sb[:, s * FLO + fo : s * FLO + fo + 1],
                    )
                for do in range(KO):
                    py = pypool.tile([128, CT], fp32)
                    for fo in range(FLO):
                        nc.tensor.matmul(
                            py[:, :tw],
                            lhsT=w2_sb[:, s, fo, do * 128 : (do + 1) * 128],
                            rhs=h_sb[:, fo, :tw],
                            start=(fo == 0),
                            stop=(fo == FLO - 1),
                        )
                    y_do = ypool.tile([128, CT], bf16, tag="y_do")
                    nc.vector.tensor_copy(y_do[:, :tw], py[:, :tw])
                    # Full-width DMA: contiguous rows (128 descriptors, no
                    # strided slow path); pad columns carry ignored stale
                    # data. Per-do DMAs pipeline under the remaining mm2s,
                    # so the kernel tail only waits on one 128 KB transfer.
                    nc.sync.dma_start(y[ti][:, do, :], y_do[:])

    nc.compile()
    return nc, spec


def kernel(x, gate_w, w1, b1, w2, b2):
    from concourse.bass_utils import run_bass_kernel_spmd

    global LAST_RESULTS

    x = np.asarray(x, dtype=np.float32)
    gate_w = np.asarray(gate_w, dtype=np.float32)
    w1 = np.asarray(w1, dtype=np.float32)
    b1 = np.asarray(b1, dtype=np.float32)
    w2 = np.asarray(w2, dtype=np.float32)
    b2 = np.asarray(b2, dtype=np.float32)

    B, S, Din = x.shape
    assert Din == D and gate_w.shape == (D, E)
    T = B * S
    xf = x.reshape(T, D)

    # ---- Host router + dispatch (as v1) ----
    logits = xf.astype(np.float64) @ gate_w.astype(np.float64)
    idx0 = np.argmax(logits, axis=1)
    rows = np.arange(T)
    v0 = logits[rows, idx0]
    l2 = logits.copy()
    l2[rows, idx0] = -np.inf
    idx1 = np.argmax(l2, axis=1)
    v1_ = l2[rows, idx1]
    e1 = np.exp(v1_ - v0)
    cw0 = 1.0 / (1.0 + e1)
    cw1 = e1 / (1.0 + e1)

    token_ids = []
    combine_w = []
    for e in range(E):
        sel0 = idx0 == e
        sel1 = idx1 == e
        ids = np.nonzero(sel0 | sel1)[0]
        w = np.where(sel0[ids], cw0[ids], cw1[ids])
        token_ids.append(ids)
        combine_w.append(w)

    counts = np.array([len(ids) for ids in token_ids])
    # Pair i-th largest with i-th smallest; slot A = the large expert.
    order = np.argsort(-counts)
    pairs = [(int(order[i]), int(order[E - 1 - i])) for i in range(E // 2)]
    CA = int(max(counts[eA] for eA, _ in pairs))
    CB = int(max(counts[eB] for _, eB in pairs))
    CA += CA & 1
    CB += CB & 1

    if (CA, CB) not in _NC_CACHE:
        _NC_CACHE[(CA, CB)] = _build(CA, CB)
    nc, spec = _NC_CACHE[(CA, CB)]
    n_tiles = len(spec)

    # ---- Per-pair token tiles; per-core weight halves ----
    in_maps = [None] * N_CORES
    pair_x = []
    for pi, (eA, eB) in enumerate(pairs):
        xtiles = np.zeros((n_tiles, 128, KO, CT), dtype=BF16)
        for ti, (s, off, tw) in enumerate(spec):
            e = (eA, eB)[s]
            ids_seg = token_ids[e][off : off + tw]
            w_val = len(ids_seg)
            if w_val == 0:
                continue
            blk = (
                xf[ids_seg].astype(BF16).reshape(w_val, KO, 128).transpose(2, 1, 0)
            )
            xtiles[ti, :, :, :w_val] = blk
        xtiles = np.ascontiguousarray(xtiles)
        pair_x.append(xtiles)
        for h in range(2):
            sl = slice(h * FL, (h + 1) * FL)
            w1c = np.stack(
                [
                    w1[e][:, sl]
                    .reshape(KO, 128, 4, 512)
                    .transpose(2, 1, 0, 3)
                    for e in (eA, eB)
                ]
            ).astype(BF16)  # [2, 4, 128, KO, 512]
            w2c = np.stack(
                [
                    w2[e][sl, :]
                    .reshape(2, 8, 128, D)
                    .transpose(0, 2, 1, 3)
                    for e in (eA, eB)
                ]
            ).astype(BF16)  # [2, 2, 128, 8, D]
            b1c = np.concatenate(
                [b1[e][sl].reshape(FLO, 128).T for e in (eA, eB)], axis=1
            )  # [128, 2*FLO]
            in_maps[2 * pi + h] = {
                "x": xtiles,
                "w1": np.ascontiguousarray(w1c),
                "w2": np.ascontiguousarray(w2c),
                "b1": np.ascontiguousarray(b1c),
            }

    res = run_bass_kernel_spmd(nc, in_maps, core_ids=list(range(N_CORES)))
    LAST_RESULTS = res

    # ---- Host: sum the pair halves, combine, scatter ----
    out = np.zeros((T, D), dtype=np.float32)
    for pi, (eA, eB) in enumerate(pairs):
        ysum = res.results[2 * pi]["y"].astype(np.float32) + res.results[
            2 * pi + 1
        ]["y"].astype(np.float32)
        for ti, (s, off, tw) in enumerate(spec):
            e = (eA, eB)[s]
            ids_seg = token_ids[e][off : off + tw]
            w_val = len(ids_seg)
            if w_val == 0:
                continue
            cw_seg = combine_w[e][off : off + w_val].astype(np.float32)
            yt = ysum[ti, :, :, :w_val].transpose(2, 1, 0).reshape(w_val, D)
            out[ids_seg] += cw_seg[:, None] * (yt + b2[e])

    return out.reshape(B, S, D)
